# revision 24
# baseline (speedup 1.0000x reference)
"""AdaptiveGraphAttention Trainium2 kernel (8 NeuronCores, data-parallel).

Math: in the reference, logits[b,h,i,j] = a_q[b,h,i] + a_k[b,h,j] +
e_j[b,h,j]*adj[i,j] + attn_b with adj[:,0]=0, adj[:,1:]=1 — the mask and the
j-dependent terms are identical for every query row i, and the a_q/bias terms
are constant over j.  Softmax is shift-invariant, so the attention
distribution p[b,h,:] = softmax_{j>=1}(a_k + e_j) is the same for all i: the
attention matrix is rank-1 and the output is one row per batch, broadcast
over the 256 query positions.  bq/bk/attn_b cancel exactly; bv survives as
an additive constant (sum_j p_j = 1); bv and bo are folded on the host.

Per-head dots fold into small matrices:
  a_k[b,j,h] = nv[b,j,:] @ Uk[:,h],  Uk[d,h] = sum_m Wk[h*64+m, d] * w_k[m]
  e_j[b,j,h] = desc[b,j-1,:] @ Ue[:,h], Ue[h*64+m, h] = w_e(m) (else 0)

Device work per core (4 batches), all cross-core-communication-free:
  psc[h,j]    = Uk.T @ nvT[:,j] + Ue.T @ descT[:,j-1]   (PE DoubleRow fp8,
                two batches per PSUM tile to halve instruction count)
  p_b[h,:]    = softmax_j(psc)              (ACT exp+accum, DVE recip/mul)
  pT[j,(b,h)] = PE transpose (XBAR DMA transpose measured 10-20us slower
                here: it blocks in-order DMA rings and burns ACT time)
  nvbarT[d,(b,h)] = nv_b-stationary @ pT    (64 small matmuls; LDW overlaps)
  VbarT       = WvT @ nvbarT, streaming only the 8 surviving (b,h) columns
                per d'-chunk; blockdiag-select to ctxT   (DVE)
  out         = ctxT.T @ WoT -> [4, 1024]   (chunks pipeline behind the
                WoT DMA)                                          -> DMA
bv is folded into the host-side output bias (out += Wo @ bv + bo, exact
since sum_j p_j = 1).

All DRAM inputs are host-prepermuted to [128, chunk, inner] so each DMA
partition row is one contiguous run.
"""

import numpy as np
import ml_dtypes
from contextlib import ExitStack

import concourse.bass as bass
import concourse.mybir as mybir
import concourse.tile as tile
from concourse import bacc
from concourse.bass_utils import run_bass_kernel_spmd
from concourse.masks import make_identity

B, S, D, H, HD = 32, 256, 1024, 16, 64
NCORES = 8
BPC = B // NCORES  # 4 batches per core
F32 = mybir.dt.float32
BF16 = mybir.dt.bfloat16
NPBF = ml_dtypes.bfloat16
F8 = mybir.dt.float8e4
NPF8 = ml_dtypes.float8_e4m3
USCALE = 512.0  # fp8 range lift for the tiny folded U entries
DC = D // 128  # 8 chunks of the model dim
JC = S // 128  # 2 chunks of the sequence dim

_cache = {}


def _build():
    nc = bacc.Bacc("TRN2", target_bir_lowering=False, debug=False,
                   num_devices=NCORES)

    nv_ext = nc.declare_dram_parameter("nv", [BPC, 128, JC, D], BF16,
                                       isOutput=False)
    xt_ext = nc.declare_dram_parameter("xT", [BPC // 2, 128, DC, 2, 2 * S],
                                       F8, isOutput=False)
    u_ext = nc.declare_dram_parameter("U", [128, DC, 2 * H], F8,
                                      isOutput=False)
    wvt_ext = nc.declare_dram_parameter("WvT", [128, DC, DC, 128], BF16,
                                        isOutput=False)
    wot_ext = nc.declare_dram_parameter("WoT", [128, DC, D], BF16,
                                        isOutput=False)
    out_ext = nc.declare_dram_parameter("out", [BPC, D], F32, isOutput=True)

    DR = mybir.MatmulPerfMode.DoubleRow
    EXPF = mybir.ActivationFunctionType.Exp

    with tile.TileContext(nc) as tc, ExitStack() as ctx:
        wpool = ctx.enter_context(tc.tile_pool(name="w", bufs=1))
        xpool = ctx.enter_context(tc.tile_pool(name="x", bufs=4))
        smpool = ctx.enter_context(tc.tile_pool(name="sm", bufs=2))
        ps_c = ctx.enter_context(tc.tile_pool(name="ps_c", bufs=1, space="PSUM"))
        ps_tp = ctx.enter_context(tc.tile_pool(name="ps_tp", bufs=1, space="PSUM"))
        ps_nb = ctx.enter_context(tc.tile_pool(name="ps_nb", bufs=2, space="PSUM"))
        ps_vb = ctx.enter_context(tc.tile_pool(name="ps_vb", bufs=2, space="PSUM"))
        ps_o = ctx.enter_context(tc.tile_pool(name="ps_o", bufs=1, space="PSUM"))

        u_sb = wpool.tile([128, DC, 2 * H], F8)
        nc.sync.dma_start(out=u_sb[:], in_=u_ext.ap())
        ident = wpool.tile([128, 128], BF16)
        make_identity(nc, ident[:])

        # activation DMAs, pair-interleaved so per-pair compute can start
        xt_sb, nv_sb = [], []
        for pb in range(BPC // 2):
            xt = xpool.tile([128, DC, 2, 2 * S], F8)
            nc.sync.dma_start(out=xt[:], in_=xt_ext[pb])
            xt_sb.append(xt)
            for i in range(2):
                nv = xpool.tile([128, JC, D], BF16)
                nc.sync.dma_start(out=nv[:], in_=nv_ext[2 * pb + i])
                nv_sb.append(nv)

        # weights after activations; chunked so phase-2 matmuls pipeline
        # behind the arriving chunks
        wvt_sb = wpool.tile([128, DC, DC, 128], BF16)
        for cm in range(DC):
            nc.sync.dma_start(out=wvt_sb[:, cm], in_=wvt_ext[:, cm])
        wot_sb = wpool.tile([128, DC, D], BF16)
        for ck in range(DC):
            nc.sync.dma_start(out=wot_sb[:, ck], in_=wot_ext[:, ck])

        # --- phase 1: logits (batch pairs) -> softmax -> pT -> nvbar -------
        pt_sb = smpool.tile([128, JC, BPC * H], BF16)
        nvall = wpool.tile([128, DC, BPC * H], BF16)
        for pb in range(BPC // 2):
            psc = ps_c.tile([H, 2, S], F32)
            for c2 in range(DC // 2):
                pair = slice(2 * c2, 2 * c2 + 2)
                nc.tensor.matmul(psc[:, :, 1:S], u_sb[:, pair, 0:H],
                                 xt_sb[pb][:, pair, :, 1:S],
                                 start=(c2 == 0), stop=False, perf_mode=DR)
            for c2 in range(DC // 2):
                pair = slice(2 * c2, 2 * c2 + 2)
                nc.tensor.matmul(psc[:, :, 1:S], u_sb[:, pair, H:2 * H],
                                 xt_sb[pb][:, pair, :, S:2 * S - 1],
                                 start=False, stop=(c2 == DC // 2 - 1),
                                 perf_mode=DR)

            for i in range(2):
                b = 2 * pb + i
                # softmax over j; logits are O(1), no max-subtraction
                p2 = smpool.tile([H, S], BF16)
                nc.gpsimd.memset(p2[:, 0:1], 0.0)
                sumx = smpool.tile([H, 1], F32)
                nc.scalar.activation(p2[:, 1:S], psc[:, i, 1:S], EXPF,
                                     scale=1.0 / USCALE, accum_out=sumx[:])
                recip = smpool.tile([H, 1], F32)
                nc.vector.reciprocal(recip[:], sumx[:])
                nc.vector.tensor_scalar_mul(p2[:, 1:S], p2[:, 1:S], recip[:])

                # pT[j, (b,h)] via PE transpose
                for jc in range(JC):
                    tps = ps_tp.tile([128, 128], BF16)
                    nc.tensor.transpose(tps[:, 0:H],
                                        p2[:, jc * 128:(jc + 1) * 128],
                                        ident[0:H, 0:H])
                    nc.vector.tensor_copy(pt_sb[:, jc, b * H:(b + 1) * H],
                                          tps[:, 0:H])

                # nvbarT[d, (b,h)] = sum_j nv[j, d] p[j, (b,h)], d-chunked
                for cm in range(DC):
                    nb_ps = ps_nb.tile([128, H], F32)
                    for jc in range(JC):
                        nc.tensor.matmul(
                            nb_ps[:],
                            nv_sb[b][:, jc, cm * 128:(cm + 1) * 128],
                            pt_sb[:, jc, b * H:(b + 1) * H],
                            start=(jc == 0), stop=(jc == JC - 1))
                    nc.vector.tensor_copy(nvall[:, cm, b * H:(b + 1) * H],
                                          nb_ps[:])

        # --- phase 2: VbarT (full d') -> blockdiag select -> out -----------
        ctx_sb = wpool.tile([128, DC, BPC], BF16)
        oA = ps_o.tile([BPC, 512], F32)
        oB = ps_o.tile([BPC, 512], F32)
        o_ps = [oA, oB]
        def emit_out(k):
            for n2 in range(2):
                cols = slice(n2 * 512, (n2 + 1) * 512)
                nc.tensor.matmul(o_ps[n2][:], ctx_sb[:, k, :],
                                 wot_sb[:, k, cols],
                                 start=(k == 0), stop=(k == DC - 1))

        for cm in range(DC):
            # only heads {2cm, 2cm+1} of this d'-chunk survive the blockdiag
            # select, so stream just their 8 (b,h) columns
            vb = ps_vb.tile([128, BPC * 2], F32)
            mov = nvall[:, :, :].rearrange(
                "p c (b h) -> p c b h", h=H)[:, :, :, 2 * cm:2 * cm + 2]
            for ck in range(DC):
                nc.tensor.matmul(vb[:], wvt_sb[:, cm, ck, :],
                                 mov[:, ck],
                                 start=(ck == 0), stop=(ck == DC - 1))
            for half in range(2):
                rows = slice(64 * half, 64 * half + 64)
                s_ap = vb[rows, :].rearrange("p (b h) -> p b h", h=2)[:, :, half]
                nc.vector.tensor_copy(ctx_sb[rows, cm, :], s_ap)
            # software pipeline: out chunk cm-1 is emitted after Vbar chunk
            # cm, so the PE has vb work in flight while the DVE select for
            # cm completes (a PE->DVE->PE chain per chunk otherwise inserts
            # ~0.5us of semaphore latency, serializing the whole tail)
            if cm > 0:
                emit_out(cm - 1)
        emit_out(DC - 1)
        o_sb = smpool.tile([BPC, D], F32)
        nc.vector.tensor_copy(o_sb[:, 0:512], o_ps[0][:])
        nc.scalar.activation(o_sb[:, 512:1024], o_ps[1][:],
                             mybir.ActivationFunctionType.Copy)
        nc.sync.dma_start(out=out_ext.ap(), in_=o_sb[:])

    nc.compile()
    return nc


def _prep(desc, nv, Wk, Wv, Wo, attn_w):
    w_k = attn_w[HD:2 * HD]
    w_e = attn_w[2 * HD:]
    Uk = np.einsum('hmd,m->dh', Wk.reshape(H, HD, D), w_k)
    Ue = np.zeros((D, H), np.float32)
    for h in range(H):
        Ue[h * HD:(h + 1) * HD, h] = w_e
    U = np.concatenate([Uk, Ue], axis=1) * USCALE           # [D, 32]
    Up = np.ascontiguousarray(
        U.reshape(DC, 128, 2 * H).swapaxes(0, 1)).astype(NPF8)
    WvTp = np.ascontiguousarray(
        Wv.T.reshape(DC, 128, DC, 128).transpose(1, 2, 0, 3)).astype(NPBF)
    WoTp = np.ascontiguousarray(
        Wo.T.reshape(DC, 128, D).swapaxes(0, 1)).astype(NPBF)
    # nv natural, chunked over j: [B, 128, JC, D]
    nvp = np.ascontiguousarray(
        nv.reshape(B, JC, 128, D).swapaxes(1, 2)).astype(NPBF)
    # nv transposed, chunked over d: [B, 128, DC, S]
    nvTp = nv.transpose(0, 2, 1).reshape(B, DC, 128, S).swapaxes(1, 2)
    descTp = desc.transpose(0, 2, 1).reshape(B, DC, 128, S - 1).swapaxes(1, 2)
    pad = np.zeros((B, 128, DC, 1), np.float32)
    xTp = np.concatenate([nvTp, descTp, pad], axis=3).astype(NPF8)
    # batch pairs side by side in the free dim: [B/2, 128, DC, 2, 2S]
    xTp = np.ascontiguousarray(
        xTp.reshape(B // 2, 2, 128, DC, 2 * S).transpose(0, 2, 3, 1, 4))
    return Up, WvTp, WoTp, nvp, xTp


def kernel(desc_embeddings, name_value_embeddings, Wq, bq, Wk, bk, Wv, bv,
           attn_w, attn_b, Wo, bo, _trace=False):
    desc = np.asarray(desc_embeddings, np.float32)
    nv = np.asarray(name_value_embeddings, np.float32)
    Up, WvTp, WoTp, nvp, xTp = _prep(
        desc, nv, np.asarray(Wk, np.float32), np.asarray(Wv, np.float32),
        np.asarray(Wo, np.float32), np.asarray(attn_w, np.float32))

    if "nc" not in _cache:
        _cache["nc"] = _build()
    nc = _cache["nc"]

    in_maps = []
    for c in range(NCORES):
        sl = slice(c * BPC, (c + 1) * BPC)
        in_maps.append({
            "nv": np.ascontiguousarray(nvp[sl]),
            "xT": np.ascontiguousarray(xTp[c * BPC // 2:(c + 1) * BPC // 2]),
            "U": Up, "WvT": WvTp, "WoT": WoTp,
        })
    res = run_bass_kernel_spmd(nc, in_maps, core_ids=list(range(NCORES)),
                               trace=_trace)
    out_rows = np.empty((B, D), np.float32)
    for c in range(NCORES):
        out_rows[c * BPC:(c + 1) * BPC] = res.results[c]["out"]
    bo_eff = (np.asarray(bo, np.float32)
              + np.asarray(Wo, np.float32) @ np.asarray(bv, np.float32))
    out_rows += bo_eff[None, :]
    full = np.broadcast_to(out_rows[:, None, :], (B, S, D))
    if _trace:
        return np.ascontiguousarray(full), res
    return np.ascontiguousarray(full)
